# revision 1
# baseline (speedup 1.0000x reference)
"""LPKT knowledge-tracing kernel for 8x Trainium2 NeuronCores.

Data-parallel over batch: B=32 -> 4 batches per core. Per core the recurrent
state h [4, C=256, K=128] is kept in SBUF transposed as hT [K=128 partitions,
(b,c)=1024 free].  Per step:
  - gates LG from h_tilde via small PE matmuls + sigmoid (tanh folded:
    (tanh(x)+1)/2 == sigmoid(2x))
  - gamma_f preact = W4a^T-matmul over hT (PE), per-batch bias u via
    activation bias operand
  - h_new = q_e (x) LG + gamma_f * h via fused scalar_tensor_tensor with the
    q_e replication (PE rank-1 matmul from previous step) in PSUM
  - h_tilde = sum_c q_next * h_new via scalar_tensor_tensor accum_out
  - y_t accumulated into a PSUM row via ones-matmul (partition reduction)
"""

import numpy as np

B, S = 32, 128
NUM_Q, NUM_C = 10000, 256
K = 128
C = NUM_C
NCORES = 8
BL = B // NCORES  # 4 batches per core
T = S - 1  # 127 recurrence steps
QP = 32  # partitions used for the q table layout

_cache = {}


def _build(np_inputs_shapes_only=None):
    import concourse.bass as bass  # noqa: F401
    import concourse.mybir as mybir
    import concourse.tile as tile
    from concourse import bacc

    fp32 = mybir.dt.float32
    AF = mybir.ActivationFunctionType
    OP = mybir.AluOpType

    nc = bacc.Bacc()

    # ---------------- DRAM I/O ----------------
    def din(name, shape):
        return nc.dram_tensor(name, shape, fp32, kind="ExternalInput")

    d = {}
    d["eT"] = din("eT", [K, BL, S])        # e_emb gathered+transposed
    d["atT"] = din("atT", [K, BL, S])
    d["itT"] = din("itT", [K, BL, S])
    d["qA"] = None  # q rows live in DRAM, staged per step
    q_dram = nc.dram_tensor("qD", [S, BL * C], fp32, kind="ExternalInput")
    del d["qA"]
    d["a_row"] = din("a_row", [1, BL * S])
    d["h0T4"] = din("h0T4", [K, BL * C])
    for w in ["W1a", "W1b", "W2a", "W2b", "W2c", "W2d",
              "W3a", "W3b", "W3c", "W3d", "W4a", "W4b", "W4c",
              "W5a", "W5b"]:
        d[w] = din(w, [K, K])
    for w in ["w1c", "b1r", "b2r", "b3r", "b4r", "b5r", "ones1r"]:
        d[w] = din(w, [1, K])
    d["ones512"] = din("ones512", [1, 512])
    d["ones128c"] = din("ones128c", [K, 1])
    y_dram = nc.dram_tensor("y_out", [1, BL * T], fp32, kind="ExternalOutput")

    from contextlib import ExitStack

    with tile.TileContext(nc) as tc, ExitStack() as ctx:
        singles = ctx.enter_context(tc.tile_pool(name="singles", bufs=1))
        state = ctx.enter_context(tc.tile_pool(name="state", bufs=1))
        sm = ctx.enter_context(tc.tile_pool(name="sm", bufs=4))
        htp = ctx.enter_context(tc.tile_pool(name="htp", bufs=3))
        pp = ctx.enter_context(tc.tile_pool(name="pp", bufs=2, space="PSUM"))
        pq = ctx.enter_context(tc.tile_pool(name="pq", bufs=2, space="PSUM"))
        psm = ctx.enter_context(tc.tile_pool(name="psm", bufs=1, space="PSUM"))
        pyacc = ctx.enter_context(tc.tile_pool(name="pyacc", bufs=1, space="PSUM"))

        # ---------------- load everything to SBUF ----------------
        sb = {}
        for name, dt_ in d.items():
            t_ = singles.tile(list(dt_.shape), fp32, tag=name)
            nc.sync.dma_start(out=t_[:], in_=dt_[:])
            sb[name] = t_

        # collapse the ~30 input-DMA dependencies so no matmul needs >1 wait
        tc.strict_bb_all_engine_barrier()

        s_hT = state.tile([K, BL * C], fp32, tag="hT")
        nc.vector.tensor_copy(out=s_hT[:], in_=sb["h0T4"][:])

        s_gam = state.tile([K, BL * C], fp32, tag="gam")
        s_m = state.tile([K, BL * C], fp32, tag="m")

        # ---------------- precompute: allT, Z2, Z3, U4, Y5 ----------------
        # allT[k, b, s] = (all_learning).T
        p_all = pp.tile([K, BL, S], fp32, tag="pbig")
        nc.tensor.matmul(out=p_all[:], lhsT=sb["W1a"][:], rhs=sb["eT"][:],
                         start=True, stop=False)
        nc.tensor.matmul(out=p_all[:], lhsT=sb["W1b"][:], rhs=sb["atT"][:],
                         start=False, stop=False)
        nc.tensor.matmul(out=p_all[:], lhsT=sb["w1c"][:], rhs=sb["a_row"][:],
                         start=False, stop=False)
        nc.tensor.matmul(out=p_all[:], lhsT=sb["b1r"][:],
                         rhs=sb["ones512"][:, 0:512], start=False, stop=True)
        s_allT = singles.tile([K, BL, S], fp32, tag="allT")
        nc.vector.tensor_copy(out=s_allT[:], in_=p_all[:])

        def precompute_z(Wpre, Wit, Wlearn, brow, tag):
            # out[:, b, t] = lp[t]@Wpre + it[t]@Wit + learn[t]@Wlearn + b
            ptile = pp.tile([K, BL, T], fp32, tag="pbig")
            nc.tensor.matmul(out=ptile[:], lhsT=sb[Wit][:],
                             rhs=sb["itT"][:, :, 0:T], start=True, stop=False)
            if Wpre is not None:
                nc.tensor.matmul(out=ptile[:, :, 1:T], lhsT=sb[Wpre][:],
                                 rhs=sb["allT"][:, :, 0:T - 1],
                                 start=False, stop=False, skip_group_check=True)
            if Wlearn is not None:
                nc.tensor.matmul(out=ptile[:], lhsT=sb[Wlearn][:],
                                 rhs=sb["allT"][:, :, 0:T], start=False, stop=False)
            nc.tensor.matmul(out=ptile[:], lhsT=sb[brow][:],
                             rhs=sb["ones512"][:, 0:BL * T], start=False, stop=True)
            s = singles.tile([K, BL, T], fp32, tag=tag)
            nc.vector.tensor_copy(out=s[:], in_=ptile[:])
            return s

        sb["allT"] = s_allT
        s_Z2 = precompute_z("W2a", "W2b", "W2c", "b2r", "Z2")
        s_Z3 = precompute_z("W3a", "W3b", "W3c", "b3r", "Z3")

        # U4[:, b, t] = it[t] @ W4c + b4
        p_u4 = pp.tile([K, BL, T], fp32, tag="pbig")
        nc.tensor.matmul(out=p_u4[:], lhsT=sb["W4c"][:],
                         rhs=sb["itT"][:, :, 0:T], start=True, stop=False)
        nc.tensor.matmul(out=p_u4[:], lhsT=sb["b4r"][:],
                         rhs=sb["ones512"][:, 0:BL * T], start=False, stop=True)
        s_U4 = singles.tile([K, BL, T], fp32, tag="U4")
        nc.vector.tensor_copy(out=s_U4[:], in_=p_u4[:])

        # Y5[:, b, t] = e_emb[t+1] @ W5a + b5
        p_y5 = pp.tile([K, BL, T], fp32, tag="pbig")
        nc.tensor.matmul(out=p_y5[:], lhsT=sb["W5a"][:],
                         rhs=sb["eT"][:, :, 1:S], start=True, stop=False)
        nc.tensor.matmul(out=p_y5[:], lhsT=sb["b5r"][:],
                         rhs=sb["ones512"][:, 0:BL * T], start=False, stop=True)
        s_Y5 = singles.tile([K, BL, T], fp32, tag="Y5")
        nc.vector.tensor_copy(out=s_Y5[:], in_=p_y5[:])

        # ---------------- q staging + replication helpers ----------------
        qstage = [None] * S

        def qstage_load(t):
            st = sm.tile([1, BL * C], fp32, tag="qstage", bufs=6)
            nc.sync.dma_start(out=st[:], in_=q_dram[t:t + 1, :])
            qstage[t] = st

        def qrep(t):
            # replicate q_t rows for all 4 batches across 128 partitions:
            # two [128, 512] psum tiles (batches 0,1 then 2,3)
            qt0 = pq.tile([K, 512], fp32, tag="q0")
            qt1 = pq.tile([K, 512], fp32, tag="q1")
            st = qstage[t]
            nc.tensor.matmul(out=qt0[:], lhsT=sb["ones1r"][:],
                             rhs=st[:, 0:512], start=True, stop=True)
            nc.tensor.matmul(out=qt1[:], lhsT=sb["ones1r"][:],
                             rhs=st[:, 512:1024], start=True, stop=True)
            return (qt0, qt1)

        def qhalf(qpair, b):
            # [128, 256] slice of the replicated q for batch b
            return qpair[b // 2][:, (b % 2) * C:(b % 2 + 1) * C]

        # ---------------- h_tilde init (with q_0) ----------------
        qstage_load(0)
        qstage_load(1)
        q_prev = qrep(0)
        ht_prev = htp.tile([K, BL], fp32, tag="ht")
        for b in range(BL):
            nc.vector.scalar_tensor_tensor(
                out=s_gam[:, 0:C], in0=s_hT[:, b * C:(b + 1) * C], scalar=0.0,
                in1=qhalf(q_prev, b), op0=OP.bypass, op1=OP.mult,
                accum_out=ht_prev[:, b:b + 1])

        p_y = pyacc.tile([1, BL * T], fp32, tag="yacc")

        # ---------------- the recurrence ----------------
        for t in range(T):
            if t + 2 < S:
                qstage_load(t + 2)
            ps = psm.tile([K, 16], fp32, tag="small")
            # gates: lg/gamma_l preacts from h_tilde
            nc.tensor.matmul(out=ps[:, 0:4], lhsT=sb["W2d"][:], rhs=ht_prev[:],
                             start=True, stop=True)
            nc.tensor.matmul(out=ps[:, 4:8], lhsT=sb["W3d"][:], rhs=ht_prev[:],
                             start=True, stop=True)
            tA = sm.tile([K, BL], fp32, tag="tA")
            tB = sm.tile([K, BL], fp32, tag="tB")
            nc.vector.tensor_add(out=tA[:], in0=ps[:, 0:4], in1=s_Z2[:, :, t])
            nc.vector.tensor_add(out=tB[:], in0=ps[:, 4:8], in1=s_Z3[:, :, t])
            s2 = sm.tile([K, BL], fp32, tag="s2")
            s3 = sm.tile([K, BL], fp32, tag="s3")
            nc.scalar.activation(out=s2[:], in_=tA[:], func=AF.Sigmoid, scale=2.0)
            nc.scalar.activation(out=s3[:], in_=tB[:], func=AF.Sigmoid)
            LGT = sm.tile([K, BL], fp32, tag="LGT")
            nc.vector.tensor_mul(out=LGT[:], in0=s2[:], in1=s3[:])

            # u = LG @ W4b + U4[t]
            nc.tensor.matmul(out=ps[:, 8:12], lhsT=sb["W4b"][:], rhs=LGT[:],
                             start=True, stop=True)
            uT = sm.tile([K, BL], fp32, tag="uT")
            nc.vector.tensor_add(out=uT[:], in0=ps[:, 8:12], in1=s_U4[:, :, t])

            # gamma_f preact (big matmul over hT)
            pP0 = pp.tile([K, 512], fp32, tag="pbig")
            pP1 = pp.tile([K, 512], fp32, tag="pbig")
            nc.tensor.matmul(out=pP0[:], lhsT=sb["W4a"][:], rhs=s_hT[:, 0:512],
                             start=True, stop=True)
            nc.tensor.matmul(out=pP1[:], lhsT=sb["W4a"][:], rhs=s_hT[:, 512:1024],
                             start=True, stop=True)
            for b in range(BL):
                src = (pP0 if b < 2 else pP1)[:, (b % 2) * C:(b % 2 + 1) * C]
                nc.scalar.activation(out=s_gam[:, b * C:(b + 1) * C], in_=src,
                                     func=AF.Sigmoid, bias=uT[:, b:b + 1])

            # m = gamma * h ; h_new = q_e_rep * LG + m   (q_e_rep from prev step)
            nc.vector.tensor_mul(out=s_m[:], in0=s_gam[:], in1=s_hT[:])
            for b in range(BL):
                nc.vector.scalar_tensor_tensor(
                    out=s_hT[:, b * C:(b + 1) * C], in0=qhalf(q_prev, b),
                    scalar=LGT[:, b:b + 1], in1=s_m[:, b * C:(b + 1) * C],
                    op0=OP.mult, op1=OP.add)

            # replicate q_{t+1}; h_tilde_new = sum_c q_next * h_new
            q_next = qrep(t + 1)
            ht_new = htp.tile([K, BL], fp32, tag="ht")
            for b in range(BL):
                nc.vector.scalar_tensor_tensor(
                    out=s_gam[:, b * C:(b + 1) * C],
                    in0=s_hT[:, b * C:(b + 1) * C], scalar=0.0,
                    in1=qhalf(q_next, b), op0=OP.bypass, op1=OP.mult,
                    accum_out=ht_new[:, b:b + 1])

            # y_t = sigmoid(e_next@W5a + h_tilde@W5b + b5) summed over K
            nc.tensor.matmul(out=ps[:, 12:16], lhsT=sb["W5b"][:], rhs=ht_new[:],
                             start=True, stop=True)
            tY = sm.tile([K, BL], fp32, tag="tY")
            nc.vector.tensor_add(out=tY[:], in0=ps[:, 12:16], in1=s_Y5[:, :, t])
            sY = sm.tile([K, BL], fp32, tag="sY")
            nc.scalar.activation(out=sY[:], in_=tY[:], func=AF.Sigmoid)
            nc.tensor.matmul(out=p_y[0:1, 4 * t:4 * t + 4], lhsT=sb["ones128c"][:],
                             rhs=sY[:], start=True, stop=True)

            q_prev = q_next
            ht_prev = ht_new

        s_y = singles.tile([1, BL * T], fp32, tag="yout")
        nc.vector.tensor_copy(out=s_y[:], in_=p_y[:])
        nc.sync.dma_start(out=y_dram[:], in_=s_y[:])

    nc.compile()
    return nc


def _prep_inputs(inputs):
    """Host-side sharding + layout prep. Returns per-core input dicts."""
    f32 = np.float32
    e_idx = np.asarray(inputs["e_data"]).astype(np.int64)
    at_idx = np.asarray(inputs["at_data"]).astype(np.int64)
    it_idx = np.asarray(inputs["it_data"]).astype(np.int64)
    a_data = np.asarray(inputs["a_data"], dtype=f32)
    q_matrix = np.asarray(inputs["q_matrix"], dtype=f32)
    e_E = np.asarray(inputs["e_E"], dtype=f32)
    at_E = np.asarray(inputs["at_E"], dtype=f32)
    it_E = np.asarray(inputs["it_E"], dtype=f32)
    W1 = np.asarray(inputs["W1"], dtype=f32)
    W2 = np.asarray(inputs["W2"], dtype=f32)
    W3 = np.asarray(inputs["W3"], dtype=f32)
    W4 = np.asarray(inputs["W4"], dtype=f32)
    W5 = np.asarray(inputs["W5"], dtype=f32)
    h0 = np.asarray(inputs["h0"], dtype=f32)

    shared = {
        "W1a": W1[0:K], "W1b": W1[K:2 * K],
        "w1c": W1[2 * K:].sum(0)[None, :] .astype(f32),
        "b1r": np.asarray(inputs["b1"], dtype=f32)[None, :],
        "W2a": W2[0:K], "W2b": W2[K:2 * K], "W2c": W2[2 * K:3 * K], "W2d": W2[3 * K:],
        "b2r": np.asarray(inputs["b2"], dtype=f32)[None, :],
        "W3a": W3[0:K], "W3b": W3[K:2 * K], "W3c": W3[2 * K:3 * K], "W3d": W3[3 * K:],
        "b3r": np.asarray(inputs["b3"], dtype=f32)[None, :],
        "W4a": W4[0:K], "W4b": W4[K:2 * K], "W4c": W4[2 * K:],
        "b4r": np.asarray(inputs["b4"], dtype=f32)[None, :],
        "W5a": W5[0:K], "W5b": W5[K:],
        "b5r": np.asarray(inputs["b5"], dtype=f32)[None, :],
        "ones1r": np.ones((1, K), f32),
        "ones512": np.ones((1, 512), f32),
        "ones128c": np.ones((K, 1), f32),
        "h0T4": np.tile(np.ascontiguousarray(h0.T), (1, BL)),
    }

    in_maps = []
    for g in range(NCORES):
        bg = slice(g * BL, (g + 1) * BL)
        e_emb = e_E[e_idx[bg]]          # [4, S, K]
        at_emb = at_E[at_idx[bg]]
        it_emb = it_E[it_idx[bg]]
        q_all = q_matrix[e_idx[bg]]     # [4, S, C]
        # qD[t, b*256 + c] = q_all[b, t, c]
        qD = np.ascontiguousarray(q_all.transpose(1, 0, 2).reshape(S, BL * C))
        m = dict(shared)
        m["eT"] = np.ascontiguousarray(e_emb.reshape(BL * S, K).T).reshape(K, BL, S)
        m["atT"] = np.ascontiguousarray(at_emb.reshape(BL * S, K).T).reshape(K, BL, S)
        m["itT"] = np.ascontiguousarray(it_emb.reshape(BL * S, K).T).reshape(K, BL, S)
        m["qD"] = qD
        m["a_row"] = np.ascontiguousarray(a_data[bg].reshape(1, BL * S))
        in_maps.append({k: np.ascontiguousarray(v) for k, v in m.items()})
    return in_maps


def _run(inputs, trace=False):
    from concourse.bass_utils import run_bass_kernel_spmd

    nc = _build()
    in_maps = _prep_inputs(inputs)
    res = run_bass_kernel_spmd(nc, in_maps, core_ids=list(range(NCORES)),
                               trace=trace)
    pred = np.zeros((B, S), np.float32)
    for g in range(NCORES):
        y = res.results[g]["y_out"].reshape(T, BL)  # [t, b]
        pred[g * BL:(g + 1) * BL, 1:] = y.T / K
    return pred, res


def kernel(**inputs):
    return _run(inputs)[0]

